# revision 1
# baseline (speedup 1.0000x reference)
"""BiCEBertAttention TRN2 kernel.

Reference semantics (B=2, T=2048, C=768, H=12 heads, D=64):
  qkv = x @ Wqkv_w.T + Wqkv_b ; heads 0-5 causal attention, heads 6-11
  anti-causal attention; out = ctx @ Wo_w.T + Wo_b.

Sharding: 8 cores = 2 batches x 4 head-groups (3 heads each). Head groups
0,1 are causal; groups 2,3 anti-causal. Anti-causal cores receive the
sequence REVERSED on the host (anti-causal attention == causal attention on
the reversed sequence), so all 8 cores run one identical causal program
(SPMD). Wqkv is column-sharded; Wo is row-sharded -> each core returns a
partial [T, C] output; the host sums the 4 partials per batch.

Per-core device program (all fp32):
  phase 1: qkvT projection. lhsT = W slices (contraction C=768 on
    partitions, 6 subtiles), rhs = xT tiles. Produces qT_h/kT_h [64, 2048]
    (feature-major, what attention needs) and v [T, 195] (natural,
    3 heads x (64 dims + ones column) -- the ones column makes the AV
    matmul also produce the softmax denominator).
  phase 2: per head, per 512-wide query block J: scores computed
    TRANSPOSED, sT[tk, tq] = kT.T @ qT (K=64), only for allowed causal
    blocks; exp on ACT (scale=1/sqrt(D), no max subtraction -- scores are
    bounded ~+-5 for this problem's distribution so exp is safe in fp32);
    diagonal 128x128 blocks multiplied by a triangular 0/1 mask;
    AV matmul accumulates o^T[65, 512] over tk (row 64 = denominator);
    normalize with reciprocal + rank-1 broadcast matmul.
  phase 3: partial out = ctxT.T @ Wo_rows  (K = 192 local ctx features).
"""

import numpy as np

import concourse.bass as bass
import concourse.mybir as mybir
import concourse.tile as tile
from concourse import bacc
from concourse.bass_utils import run_bass_kernel_spmd
from concourse.masks import make_upper_triangular

B, T, C, H, D = 2, 2048, 768, 12, 64
N_LEFT = 6
HPC = 3          # heads per core
NCORES = 8
KO = C // 128    # 6 contraction subtiles
NT = T // 128    # 16 key tiles
NJ = T // 512    # 4 query blocks
VW = 256         # v width: 3 x (64 dims + ones col) = 195, padded to 256
                 # (fp32r matmuls need moving dim >= 256 for full rate)
f32 = mybir.dt.float32
f32r = mybir.dt.float32r
Exp = mybir.ActivationFunctionType.Exp

_NC_CACHE: dict = {}


def build_nc(use_pad: bool, use_bqk: bool, reps: int = 1):
    nc = bacc.Bacc("TRN2", target_bir_lowering=False, debug=False)

    xT = nc.declare_dram_parameter("xT", [C, T], f32r, isOutput=False)
    wqk = nc.declare_dram_parameter("wqk", [C, HPC * 128], f32r, isOutput=False)
    wv = nc.declare_dram_parameter("wv", [C, VW], f32r, isOutput=False)
    bqk = nc.declare_dram_parameter("bqk", [1, HPC * 128], f32r, isOutput=False)
    bv = nc.declare_dram_parameter("bv", [1, VW], f32r, isOutput=False)
    wo = nc.declare_dram_parameter("wo", [HPC * 64, C], f32r, isOutput=False)
    pad = nc.declare_dram_parameter("pad", [1, T], f32r, isOutput=False)
    out = nc.declare_dram_parameter("out", [T, C], f32, isOutput=True)

    xT_r = xT.rearrange("(ko p) t -> p ko t", p=128)
    wqk_r = wqk.rearrange("(ko p) f -> p ko f", p=128)
    wv_r = wv.rearrange("(ko p) f -> p ko f", p=128)

    with tile.TileContext(nc) as tc:
        with (
            nc.allow_low_precision(
                reason="fp32r everywhere: ~19-bit mantissa is ample here"),
            tc.tile_pool(name="const", bufs=1) as cp,
            tc.tile_pool(name="qk", bufs=1) as qkp,
            tc.tile_pool(name="vp", bufs=1) as vp,
            tc.tile_pool(name="ctx", bufs=1) as ctxp,
        ):
            # ---- constants / weights ----
            wqk_sb = cp.tile([128, KO, HPC * 128], f32r, tag="wqk")
            wv_sb = cp.tile([128, KO, VW], f32r, tag="wv")
            bqk_sb = cp.tile([1, HPC * 128], f32r, tag="bqk")
            bv_sb = cp.tile([1, VW], f32r, tag="bv")
            wo_a = cp.tile([128, C], f32r, tag="wo_a")
            wo_b = cp.tile([64, C], f32r, tag="wo_b")
            pad_sb = cp.tile([1, T], f32r, tag="pad")
            ones_f = cp.tile([128, 512], f32, tag="ones_f")
            ones_sb = cp.tile([128, 512], f32r, tag="ones")
            tri_sb = cp.tile([128, 128], f32, tag="tri")

            # ---- persistent activations ----
            qt = [qkp.tile([64, T], f32r, tag=f"qt{h}", name=f"qt{h}")
                  for h in range(HPC)]
            kt = [qkp.tile([64, T], f32r, tag=f"kt{h}", name=f"kt{h}")
                  for h in range(HPC)]
            v_sb = vp.tile([128, NT, VW], f32r, tag="v")
            ctxa = ctxp.tile([128, T], f32r, tag="ctxa")
            ctxb = ctxp.tile([64, T], f32r, tag="ctxb")

            # ---- fused J loop: qkv(J) -> attention(J, all heads) -> Wo(J).
            # Causal structure means attention block J only reads q/k/v up
            # to column (J+1)*512, so block J overlaps the projection of
            # block J+1 and the Wo of block J-1. PSUM budget (8 banks):
            # pp 1 + wo 1 + s 2x2 + o 2 = 8 (bc borrows an s slot).
            with (
                tc.tile_pool(name="xp", bufs=2) as xpool,
                tc.tile_pool(name="pp", bufs=1, space="PSUM") as pp,
                tc.tile_pool(name="wop", bufs=1, space="PSUM") as wop,
                tc.tile_pool(name="spool", bufs=2, space="PSUM") as spool,
                tc.tile_pool(name="opool", bufs=2, space="PSUM") as opool,
                tc.tile_pool(name="epool", bufs=5) as epool,
                tc.tile_pool(name="npool", bufs=2) as npool,
                tc.tile_pool(name="pout", bufs=3) as poutp,
            ):
                xp_tiles = {}

                def emit_xp_dma(Jn):
                    xpt = xpool.tile([128, KO, 512], f32r, tag="x",
                                     name=f"xp{Jn}")
                    xp_tiles[Jn] = xpt
                    for k in range(KO):
                        nc.sync.dma_start(
                            xpt[:, k, :], xT_r[:, k, Jn * 512:(Jn + 1) * 512])

                def emit_qk_chain(Jn, h):
                    xpt = xp_tiles[Jn]
                    ps = pp.tile([128, 512], f32, tag="p1", name="psqk")
                    for k in range(KO):
                        nc.tensor.matmul(
                            ps[:], wqk_sb[:, k, h * 128:(h + 1) * 128],
                            xpt[:, k, :], start=(k == 0),
                            stop=(k == KO - 1 and not use_bqk))
                    if use_bqk:
                        nc.tensor.matmul(
                            ps[:], bqk_sb[0:1, h * 128:(h + 1) * 128],
                            ones_sb[0:1, :], start=False, stop=True)
                    nc.vector.tensor_copy(
                        qt[h][:, Jn * 512:(Jn + 1) * 512], ps[0:64, :])
                    nc.vector.tensor_copy(
                        kt[h][:, Jn * 512:(Jn + 1) * 512], ps[64:128, :])

                def emit_v_chain(Jn, sub):
                    xpt = xp_tiles[Jn]
                    pv = pp.tile([128, VW], f32, tag="p1", name="psv")
                    for k in range(KO):
                        nc.tensor.matmul(
                            pv[:], xpt[:, k, sub * 128:(sub + 1) * 128],
                            wv_sb[:, k, :], start=(k == 0), stop=False)
                    nc.tensor.matmul(pv[:], ones_sb[0:1, 0:128], bv_sb[0:1, :],
                                     start=False, stop=True)
                    nc.vector.tensor_copy(v_sb[:, Jn * 4 + sub, :], pv[:])

                def emit_wo(t):
                    # ctx for this block must be complete: force-emit any
                    # deferred normalize-part-2 for blocks <= t//4
                    while pending_n2 and pending_n2[0][1] <= t // 4:
                        pending_n2.pop(0)[2]()
                    po = poutp.tile([128, C], f32, tag="po")
                    for n in range(2):
                        wps = wop.tile([128, 384], f32, tag="wo", name="pswo")
                        nc.tensor.matmul(
                            wps[:], ctxa[:, t * 128:(t + 1) * 128],
                            wo_a[:, n * 384:(n + 1) * 384],
                            start=True, stop=False)
                        nc.tensor.matmul(
                            wps[:], ctxb[:, t * 128:(t + 1) * 128],
                            wo_b[:, n * 384:(n + 1) * 384],
                            start=False, stop=True)
                        nc.vector.tensor_copy(po[:, n * 384:(n + 1) * 384],
                                              wps[:])
                    nc.sync.dma_start(out[t * 128:(t + 1) * 128, :], po[:])

                # timing harness: `reps` repeats the computation
                # back-to-back inside one NEFF
                for _rep in range(reps):
                    if _rep == 0:
                        # loads ordered by first use: interleave W_qk and x
                        # per contraction subtile so chain k can start as
                        # soon as slice k lands
                        xpt0 = xpool.tile([128, KO, 512], f32r, tag="x",
                                          name="xp0")
                        xp_tiles[0] = xpt0
                        for k in range(KO):
                            nc.sync.dma_start(wqk_sb[:, k, :], wqk_r[:, k, :])
                            nc.sync.dma_start(xpt0[:, k, :],
                                              xT_r[:, k, 0:512])
                        for k in range(KO):
                            nc.gpsimd.dma_start(wv_sb[:, k, :], wv_r[:, k, :])
                        nc.gpsimd.dma_start(bqk_sb[:], bqk[:])
                        nc.gpsimd.dma_start(bv_sb[:], bv[:])
                        if use_pad:
                            nc.gpsimd.dma_start(pad_sb[:], pad[:])
                        nc.vector.memset(ones_f[:], 1.0)
                        nc.vector.tensor_copy(ones_sb[:], ones_f[:])
                        make_upper_triangular(nc, tri_sb[:], val=1.0,
                                              diag=True)
                        nc.gpsimd.dma_start(wo_a[:], wo[0:128, :])
                        nc.gpsimd.dma_start(wo_b[:], wo[128:192, :])
                    else:
                        emit_xp_dma(0)
                    emit_qk_chain(0, 0)
                    for sub in range(4):
                        emit_v_chain(0, sub)

                    pending_n2 = []
                    gtick = 0
                    for J in range(NJ):
                        # Filler work interleaved into this block's attention
                        # stream: next block's projection + previous block's Wo.
                        # wo tiles of earlier blocks are deferred toward
                        # the late (ACT-bound) blocks to feed the idle PE
                        wo_sched = {0: [], 1: [0], 2: [], 3: [1, 2]}
                        fillers = []
                        if J == 0:
                            # block-0 projections for heads 1,2 (head 0 and v
                            # are in the prologue); must pop before those
                            # heads' first sT, which stride-1 popping at
                            # ticks 1,2 guarantees (h1 starts at tick 2)
                            fillers.append(lambda: emit_qk_chain(0, 1))
                            fillers.append(lambda: emit_qk_chain(0, 2))
                        if J + 1 < NJ:
                            fillers.append(lambda Jn=J + 1: emit_xp_dma(Jn))
                            for h in range(HPC):
                                fillers.append(
                                    lambda Jn=J + 1, hh=h: emit_qk_chain(Jn, hh))
                            for sub in range(4):
                                fillers.append(
                                    lambda Jn=J + 1, ss=sub: emit_v_chain(Jn, ss))
                        for Jw in wo_sched[J]:
                            for sub in range(4):
                                fillers.append(
                                    lambda tt=Jw * 4 + sub: emit_wo(tt))

                        nrows = 4 * J + 4
                        npairs = nrows // 2
                        ticks = HPC * npairs
                        stride = max(1, ticks // max(1, len(fillers)))
                        tick = 0

                        def do_av(item, h, nrows):
                            eTq, rowsq = item
                            for idx, tkr in enumerate(rowsq):
                                off = max(0, (tkr - 4 * J) * 128)
                                nc.tensor.matmul(
                                    o_ps[:, off:512],
                                    v_sb[:, tkr, h * 65:(h + 1) * 65],
                                    eTq[:, idx * 512 + off:(idx + 1) * 512],
                                    start=(tkr == 0), stop=(tkr == nrows - 1))

                        for h in range(HPC):
                            o_ps = opool.tile([65, 512], f32, tag="o")
                            av_q = []
                            for pr in range(npairs):
                                rows = (2 * pr, 2 * pr + 1)
                                s_ps = spool.tile([128, 1024], f32, tag="s")
                                eT = epool.tile([128, 1024], f32r, tag="e")
                                for idx, tkr in enumerate(rows):
                                    off = max(0, (tkr - 4 * J) * 128)
                                    n0 = idx * 512 + off
                                    n1 = (idx + 1) * 512
                                    nc.tensor.matmul(
                                        s_ps[:, n0:n1],
                                        kt[h][:, tkr * 128:(tkr + 1) * 128],
                                        qt[h][:, J * 512 + off:(J + 1) * 512],
                                        start=True, stop=not use_pad)
                                    if use_pad:
                                        nc.tensor.matmul(
                                            s_ps[:, n0:n1],
                                            pad_sb[0:1, tkr * 128:(tkr + 1) * 128],
                                            ones_sb[0:1, 0:512 - off],
                                            start=False, stop=True)
                                if rows[0] < 4 * J:
                                    # both rows full: one exp over the pair
                                    nc.scalar.activation(eT[:], s_ps[:], Exp,
                                                         scale=0.125)
                                else:
                                    for idx, tkr in enumerate(rows):
                                        off = (tkr - 4 * J) * 128
                                        n0 = idx * 512 + off
                                        n1 = (idx + 1) * 512
                                        nc.scalar.activation(
                                            eT[:, n0:n1], s_ps[:, n0:n1], Exp,
                                            scale=0.125)
                                for idx, tkr in enumerate(rows):
                                    if tkr >= 4 * J:  # diagonal block mask
                                        n0 = idx * 512 + (tkr - 4 * J) * 128
                                        nc.gpsimd.tensor_tensor(
                                            eT[:, n0:n0 + 128], eT[:, n0:n0 + 128],
                                            tri_sb[:], mybir.AluOpType.mult)
                                av_q.append((eT, rows))
                                if len(av_q) > 3:
                                    do_av(av_q.pop(0), h, nrows)
                                # run deferred normalize-part-2 only after
                                # the reciprocal has had a few ticks, so the
                                # bc matmul never stalls the in-order PE
                                # stream
                                if pending_n2 and gtick >= pending_n2[0][0]:
                                    pending_n2.pop(0)[2]()
                                tick += 1
                                gtick += 1
                                if tick % stride == 0 and fillers:
                                    fillers.pop(0)()
                            for item in av_q:
                                do_av(item, h, nrows)
                            # normalize part 1: reciprocal of denom + evacuate o
                            rt = npool.tile([65, 512], f32r, tag="rt")
                            nc.vector.reciprocal(rt[64:65, :], o_ps[64:65, :])
                            tmp = npool.tile([64, 512], f32, tag="tmp")
                            nc.vector.tensor_copy(tmp[:], o_ps[0:64, :])

                            # normalize part 2 (deferred into the next stream):
                            # bcast matmul + multiply into ctx
                            def n2(h=h, J=J, rt=rt, tmp=tmp):
                                bc = spool.tile([64, 512], f32, tag="s", name="bc")
                                nc.tensor.matmul(bc[:], ones_sb[64:65, 0:64],
                                                 rt[64:65, :], start=True,
                                                 stop=True)
                                dst = (ctxa[64 * h:64 * h + 64,
                                            J * 512:(J + 1) * 512]
                                       if h < 2 else ctxb[:, J * 512:(J + 1) * 512])
                                nc.vector.tensor_tensor(dst, tmp[:], bc[:],
                                                        mybir.AluOpType.mult)
                            pending_n2.append((gtick + 6, J, n2))

                        for f in fillers:  # flush leftovers
                            f()
                    for _, _, f in pending_n2:
                        f()
                    pending_n2 = []
                    for sub in range(4):
                        emit_wo((NJ - 1) * 4 + sub)

    nc.finalize()
    return nc


def _get_nc(use_pad: bool, use_bqk: bool, reps: int = 1):
    key = (use_pad, use_bqk, reps)
    if key not in _NC_CACHE:
        _NC_CACHE[key] = build_nc(use_pad, use_bqk, reps)
    return _NC_CACHE[key]


def _core_inputs(c, x, attention_mask, Wqkv_w, Wqkv_b, Wo_w, use_pad):
    b, g = c // 4, c % 4
    rev = g >= 2
    heads = [3 * g + i for i in range(HPC)]

    xb = x[b]
    if rev:
        xb = xb[::-1, :]
    xT = np.ascontiguousarray(xb.T, dtype=np.float32)

    wqk = np.empty((HPC * 128, C), dtype=np.float32)
    bqk = np.empty((1, HPC * 128), dtype=np.float32)
    wv = np.zeros((VW, C), dtype=np.float32)  # rows 195..255 stay zero
    bv = np.zeros((1, VW), dtype=np.float32)
    wo = np.empty((HPC * 64, C), dtype=np.float32)
    for i, hd in enumerate(heads):
        qs, ks, vs = hd * 64, C + hd * 64, 2 * C + hd * 64
        wqk[i * 128:i * 128 + 64] = Wqkv_w[qs:qs + 64]
        wqk[i * 128 + 64:(i + 1) * 128] = Wqkv_w[ks:ks + 64]
        bqk[0, i * 128:i * 128 + 64] = Wqkv_b[qs:qs + 64]
        bqk[0, i * 128 + 64:(i + 1) * 128] = Wqkv_b[ks:ks + 64]
        wv[i * 65:i * 65 + 64] = Wqkv_w[vs:vs + 64]
        bv[0, i * 65:i * 65 + 64] = Wqkv_b[vs:vs + 64]
        bv[0, i * 65 + 64] = 1.0
        wo[i * 64:(i + 1) * 64] = Wo_w[:, hd * 64:(hd + 1) * 64].T

    if use_pad:
        padv = ((1.0 - attention_mask[b].astype(np.float32)) * -30000.0)
        if rev:
            padv = padv[::-1]
        padv = np.ascontiguousarray(padv.reshape(1, T), dtype=np.float32)
    else:
        padv = np.zeros((1, T), dtype=np.float32)

    return {
        "xT": xT,
        "wqk": np.ascontiguousarray(wqk.T),
        "wv": np.ascontiguousarray(wv.T),
        "bqk": bqk,
        "bv": bv,
        "wo": np.ascontiguousarray(wo),
        "pad": padv,
    }


def run_cores(x, attention_mask, Wqkv_w, Wqkv_b, Wo_w, trace=False):
    use_pad = not bool(np.all(attention_mask == 1))
    use_bqk = bool(np.any(Wqkv_b[:2 * C] != 0.0))
    nc = _get_nc(use_pad, use_bqk)
    in_maps = [
        _core_inputs(c, x, attention_mask, Wqkv_w, Wqkv_b, Wo_w, use_pad)
        for c in range(NCORES)
    ]
    return run_bass_kernel_spmd(nc, in_maps, list(range(NCORES)), trace=trace)


def kernel(x, attention_mask, Wqkv_w, Wqkv_b, Wo_w, Wo_b):
    x = np.asarray(x, dtype=np.float32)
    attention_mask = np.asarray(attention_mask)
    Wqkv_w = np.asarray(Wqkv_w, dtype=np.float32)
    Wqkv_b = np.asarray(Wqkv_b, dtype=np.float32)
    Wo_w = np.asarray(Wo_w, dtype=np.float32)
    Wo_b = np.asarray(Wo_b, dtype=np.float32)

    res = run_cores(x, attention_mask, Wqkv_w, Wqkv_b, Wo_w)
    out = np.zeros((B, T, C), dtype=np.float32)
    for c in range(NCORES):
        b, g = c // 4, c % 4
        po = res.results[c]["out"]
        if g >= 2:
            po = po[::-1, :]
        out[b] += po
    out += Wo_b
    return out.astype(np.float32)

